# revision 1
# baseline (speedup 1.0000x reference)
"""BIDE forward kernel for Trainium2, 8-core data parallel over B.

Math: logit[b, v] = sum_h cos(zlo[b, lo(v), h] + zhi[b, hi(v), h]) where
  zlo = bits(lo) @ W[:, :8].T          (lo = v & 255)
  zhi = bits(hi) @ W[:, 8:].T + r      (hi = v >> 8)
Using cos(p+q) = cos p cos q - sin p sin q, the [256, 256] logits table is
two K=128 matmuls over trig tables of shape [128 h, 256]:
  table = CloT.T @ ChiT - SloT.T @ ShiT   (per batch row)
logZ = EXP_SHIFT + log(sum_v exp(table - EXP_SHIFT)) (constant shift: the
realized max logit is ~89, exp would overflow fp32 and the ACT Ln spline
is only valid to 2^64), and the output gather out[b, t] = table[x[b, t]]
- logZ is a per-element indirect DMA from a DRAM copy of the table.

Sin on the scalar engine only accepts [-pi, pi] (verified: it extrapolates
garbage outside), and the DVE has no mod op, so range reduction uses the
round-to-nearest f32->i32 conversion: the z matmul weights are pre-scaled
by 1/2pi so PSUM holds q = z/2pi; then qi = round(q + c'), w = q - qi, and
sin(z + 2pi c') = Sin(w; scale=2pi, bias=2pi c') with |2pi w + bias| <= pi.

Each core handles 2 of the 16 batch rows; zero cross-core communication.
"""

import numpy as np
import ml_dtypes
from contextlib import ExitStack

import concourse.bacc as bacc
import concourse.bass as bass
from concourse import mybir
from concourse.bass_utils import run_bass_kernel_spmd
from concourse.tile import TileContext

F32 = mybir.dt.float32
BF16 = mybir.dt.bfloat16
I32 = mybir.dt.int32

PI = float(np.float32(np.pi))
HALF_PI = float(np.float32(np.pi / 2.0))
TWO_PI = float(np.float32(2.0 * np.pi))
INV_2PI = 1.0 / (2.0 * np.pi)
# logits for these inputs peak at ~89 (exp overflows fp32) and the ACT Ln
# spline is only valid to 2^64; shift exp by a constant and add it back
EXP_SHIFT = 60.0

N_CORES = 8
B, H, T = 16, 128, 4096
BPC = B // N_CORES  # batch rows per core (2)


def _build():
    nc = bacc.Bacc("TRN2", target_bir_lowering=False, debug=False)

    # lhsT for the z matmuls, one 128-col group per (b, half):
    # rows 0-7 W_hi bits, 8-15 W_lo residual, 16 r_hi, 17 r_lo (hi half only)
    wp = nc.dram_tensor("wp", [18, 512], BF16, kind="ExternalInput")
    # bit-plane enumeration of v in [0, 256): rows 0-7 and 8-15 = (v>>k)&1,
    # rows 16-17 = 1.0 (carries r into zhi)
    bits = nc.dram_tensor("bits", [18, 256], BF16, kind="ExternalInput")
    # x indices for the gather: block b at cols [32b, 32b+32), laid out so
    # the indirect DMA's partition-major offset walk (i = s*128 + p) visits
    # t in order: xv[p, 32b + s] = x[b, 128s + p]
    xv = nc.dram_tensor("xv", [128, 64], I32, kind="ExternalInput")
    # negsel[k, 128b + m] = -1 if k == b else 0  (broadcast of -logZ_b)
    negsel_in = nc.dram_tensor("negsel", [2, 256], F32, kind="ExternalInput")
    out = nc.dram_tensor("out", [BPC, T], F32, kind="ExternalOutput")

    with ExitStack() as ctx:
        tc = ctx.enter_context(TileContext(nc))
        sb = ctx.enter_context(tc.tile_pool(name="sb", bufs=1))
        ps_z = ctx.enter_context(tc.tile_pool(name="ps_z", bufs=2, space="PSUM"))
        ps_t = ctx.enter_context(tc.tile_pool(name="ps_t", bufs=2, space="PSUM"))
        ps_s = ctx.enter_context(tc.tile_pool(name="ps_s", bufs=1, space="PSUM"))
        dram = ctx.enter_context(tc.tile_pool(name="dram", bufs=1, space="DRAM"))

        # ---- input loads
        wp_sb = sb.tile([18, 512], BF16, tag="wp")
        bits_sb = sb.tile([18, 256], BF16, tag="bits")
        xv_sb = sb.tile([128, 64], I32, tag="xv")
        nc.sync.dma_start(out=wp_sb[:], in_=wp[:])
        nc.sync.dma_start(out=bits_sb[:], in_=bits[:])
        nc.sync.dma_start(out=xv_sb[:], in_=xv[:])

        # ---- constants
        ones = sb.tile([128, 1], F32, tag="ones")
        nc.vector.memset(ones[:], 1.0)
        # per-partition bias tile for Sin (const-AP registry only has 0/1)
        pio2 = sb.tile([128, 1], F32, tag="pio2")
        nc.vector.memset(pio2[:], HALF_PI)
        neg_shift = sb.tile([128, 1], F32, tag="neg_shift")
        nc.vector.memset(neg_shift[:], -EXP_SHIFT)
        negsel = sb.tile([2, 256], F32, tag="negsel")
        nc.sync.dma_start(out=negsel[:], in_=negsel_in[:])

        # ---- q matmuls: q = z/2pi (weights pre-scaled by 1/2pi), [b0|b1]
        qlo_ps = ps_z.tile([128, 512], F32, tag="z")
        qhi_ps = ps_z.tile([128, 512], F32, tag="z")
        for b in range(BPC):
            nc.tensor.matmul(
                out=qlo_ps[:, 256 * b : 256 * b + 256],
                lhsT=wp_sb[:, 128 * (2 * b) : 128 * (2 * b) + 128],
                rhs=bits_sb[:],
                start=True,
                stop=True,
            )
            nc.tensor.matmul(
                out=qhi_ps[:, 256 * b : 256 * b + 256],
                lhsT=wp_sb[:, 128 * (2 * b + 1) : 128 * (2 * b + 1) + 128],
                rhs=bits_sb[:],
                start=True,
                stop=True,
            )

        # ---- range reduction + trig, per batch row so b0's table chain
        # starts as early as possible: qi = round(q + c') (f32->i32 rounds
        # to nearest), w = q - qi, then Sin(scale*w + bias), |arg| <= pi
        F32R = mybir.dt.float32r
        t_a = sb.tile([128, 512], F32R, tag="t_a")  # cos(zlo)
        t_b = sb.tile([128, 512], F32R, tag="t_b")  # cos(zhi)
        t_c = sb.tile([128, 512], F32R, tag="t_c")  # sin(zlo)
        t_d = sb.tile([128, 512], F32R, tag="t_d")  # -sin(zhi) (scale=-2pi)

        def trig_for_b(b):
            bs = slice(256 * b, 256 * b + 256)
            for i, (t_t, q_ps, cp, scale, bias) in enumerate((
                (t_a, qlo_ps, 0.25, TWO_PI, None),
                (t_b, qhi_ps, 0.25, TWO_PI, None),
                (t_c, qlo_ps, 0.0, TWO_PI, 0.0),
                (t_d, qhi_ps, 0.0, -TWO_PI, 0.0),
            )):
                qi_t = sb.tile([128, 256], I32, tag=f"qi{i}{b}")
                if cp == 0.0:
                    nc.vector.tensor_copy(out=qi_t[:], in_=q_ps[:, bs])
                else:
                    nc.vector.tensor_scalar(
                        out=qi_t[:], in0=q_ps[:, bs], scalar1=cp, scalar2=None,
                        op0=mybir.AluOpType.add,
                    )
                w_t = sb.tile([128, 256], F32, tag=f"w{i}{b}")
                nc.vector.tensor_tensor(
                    out=w_t[:], in0=q_ps[:, bs], in1=qi_t[:],
                    op=mybir.AluOpType.subtract,
                )
                nc.scalar.activation(
                    out=t_t[:, bs], in_=w_t[:],
                    func=mybir.ActivationFunctionType.Sin,
                    bias=bias if isinstance(bias, float) else pio2[:],
                    scale=scale,
                )

        # ---- per-b pipeline: table matmuls -> copy/DMA + exp/sum -> gather
        tb_ps = []
        tb_sb = []
        e_sb = []
        g_sb = []
        tbl_dram = []
        sums2 = sb.tile([128, 2], F32, tag="sums2")
        for b in range(BPC):
            trig_for_b(b)
            bs = slice(256 * b, 256 * b + 256)
            t_ps = ps_t.tile([128, 512], F32, tag="tb")
            tb_ps.append(t_ps)
            # table[hi, lo] = sum_h cos(zhi)cos(zlo) - sin(zhi)sin(zlo)
            #   = A.T@B pairing: lhsT 128-col chunk of hi, rhs full 256 lo
            for c in range(2):
                cs = slice(256 * c, 256 * c + 256)
                hi_s = slice(256 * b + 128 * c, 256 * b + 128 * c + 128)
                nc.tensor.matmul(
                    out=t_ps[:, cs],
                    lhsT=t_b[:, hi_s],
                    rhs=t_a[:, bs],
                    start=True, stop=False,
                )
                nc.tensor.matmul(
                    out=t_ps[:, cs],
                    lhsT=t_d[:, hi_s],
                    rhs=t_c[:, bs],
                    start=False, stop=True,
                )
            # raw table to SBUF (DMA cannot read PSUM), then to DRAM
            t_sb = sb.tile([128, 512], F32, tag=f"tsb{b}")
            tb_sb.append(t_sb)
            nc.vector.tensor_copy(out=t_sb[:], in_=t_ps[:])
            tbl = dram.tile([65536, 1], F32, tag=f"tbl{b}")
            tbl_dram.append(tbl)
            for c in range(2):
                dst = tbl[32768 * c : 32768 * (c + 1), 0:1].rearrange(
                    "(p n) one -> p (n one)", p=128
                )
                nc.sync.dma_start(out=dst, in_=t_sb[:, 256 * c : 256 * c + 256])
            # gather: a [1, N, 1] dest makes the DGE emit one descriptor per
            # element, walking the offset AP partition-major (i = 128s + p).
            # One dest partition = one SBUF port (~6.4ns/element serialized),
            # so split into 4 calls on partitions {0,4,8,12} (+16 for b1),
            # which map to 8 distinct SBUF ports across the two batch rows.
            g_t = sb.tile([32, 1024], F32, tag=f"g{b}")
            for c in range(4):
                row = 4 * c + 16 * b
                nc.gpsimd.indirect_dma_start(
                    out=g_t[row : row + 1, :].rearrange(
                        "one (i x) -> one i x", x=1
                    ),
                    out_offset=None,
                    in_=tbl[:],
                    in_offset=bass.IndirectOffsetOnAxis(
                        ap=xv_sb[:, 32 * b + 8 * c : 32 * b + 8 * c + 8], axis=0
                    ),
                )
            # redistribute to g2[p, j] = value for t = 32p + j: with p =
            # 32c + q, t = 1024c + 32q + j lives at g_t[4c + 16b, 32q + j]
            g2_t = sb.tile([128, 32], F32, tag=f"g2{b}")
            g_sb.append(g2_t)
            for c in range(4):
                nc.sync.dma_start(
                    out=g2_t[32 * c : 32 * c + 32, :],
                    in_=g_t[4 * c + 16 * b : 4 * c + 16 * b + 1, :].rearrange(
                        "one (q j) -> one q j", j=32
                    ),
                )
            # exp + row sums for the partition function; EXP_SHIFT keeps
            # exp and the ln input inside fp32 / ACT-spline range
            e_t = sb.tile([128, 512], F32, tag=f"e{b}")
            e_sb.append(e_t)
            nc.scalar.activation(
                out=e_t[:], in_=t_ps[:], func=mybir.ActivationFunctionType.Exp,
                bias=neg_shift[:],
            )
            nc.vector.reduce_sum(
                sums2[:, b : b + 1], e_t[:], axis=mybir.AxisListType.X
            )

        # ---- logZ_b = ln(sum_v exp): partition sum via ones-matmul
        s_ps = ps_s.tile([2, 1], F32, tag="sps")
        nc.tensor.matmul(out=s_ps[:], lhsT=sums2[:], rhs=ones[:], start=True, stop=True)
        logz2 = sb.tile([2, 1], F32, tag="logz2")
        nc.scalar.activation(
            out=logz2[:], in_=s_ps[:], func=mybir.ActivationFunctionType.Ln,
        )

        # ---- out[b, t] = gathered - logZ_b, broadcast via negsel matmul
        for b in range(BPC):
            nz_ps = ps_s.tile([128, 1], F32, tag=f"nz{b}")
            nc.tensor.matmul(
                out=nz_ps[:],
                lhsT=negsel[:, 128 * b : 128 * b + 128],
                rhs=logz2[:],
                start=True, stop=True,
            )
            nz_sb = sb.tile([128, 1], F32, tag=f"nz{b}")
            nc.vector.tensor_copy(out=nz_sb[:], in_=nz_ps[:])
            o_t = sb.tile([128, 32], F32, tag=f"o{b}")
            nc.vector.tensor_scalar(
                out=o_t[:], in0=g_sb[b][:], scalar1=nz_sb[:], scalar2=-EXP_SHIFT,
                op0=mybir.AluOpType.add, op1=mybir.AluOpType.add,
            )
            dst = out[b, :].rearrange("(p j) -> p j", p=128)
            nc.sync.dma_start(out=dst, in_=o_t[:])

    nc.finalize()
    return nc


_NC = None


def _get_nc():
    global _NC
    if _NC is None:
        _NC = _build()
    return _NC


def _bf16_split(a):
    """Return (hi, lo) bf16 arrays with hi + lo ~= a (fp32)."""
    hi = a.astype(ml_dtypes.bfloat16)
    lo = (a - hi.astype(np.float32)).astype(ml_dtypes.bfloat16)
    return hi, lo


def _make_in_maps(x, W, r):
    x = np.asarray(x, dtype=np.int32)
    W = np.asarray(W, dtype=np.float32)
    r = np.asarray(r, dtype=np.float32)

    v = np.arange(256, dtype=np.int32)
    k = np.arange(8, dtype=np.int32)
    bitplanes = ((v[None, :] >> k[:, None]) & 1).astype(np.float32)  # [8, 256]
    bits = np.ones((18, 256), dtype=np.float32)
    bits[0:8] = bitplanes
    bits[8:16] = bitplanes
    bits = bits.astype(ml_dtypes.bfloat16)

    negsel = np.zeros((2, 256), dtype=np.float32)
    negsel[0, 0:128] = -1.0
    negsel[1, 128:256] = -1.0

    in_maps = []
    for core in range(N_CORES):
        wp = np.zeros((18, 512), dtype=ml_dtypes.bfloat16)
        xvs = []
        for b_loc in range(BPC):
            b = BPC * core + b_loc
            for half in range(2):
                g = 2 * b_loc + half
                cs = slice(128 * g, 128 * g + 128)
                w_t = W[b, :, 8 * half : 8 * half + 8].T * INV_2PI  # [8, 128]
                w_hi, w_lo = _bf16_split(w_t.astype(np.float32))
                wp[0:8, cs] = w_hi
                wp[8:16, cs] = w_lo
                if half == 1:
                    r_hi, r_lo = _bf16_split((r[b] * INV_2PI).astype(np.float32))
                    wp[16, cs] = r_hi
                    wp[17, cs] = r_lo
            xvs.append(x[b].reshape(32, 128).T)
        in_maps.append(
            {
                "wp": wp,
                "bits": bits,
                "xv": np.concatenate(xvs, axis=1).astype(np.int32),
                "negsel": negsel,
            }
        )
    return in_maps


def _run(x, W, r, trace=False):
    nc = _get_nc()
    in_maps = _make_in_maps(x, W, r)
    res = run_bass_kernel_spmd(nc, in_maps, core_ids=list(range(N_CORES)), trace=trace)
    out = np.concatenate([res.results[c]["out"] for c in range(N_CORES)], axis=0)
    return out.astype(np.float32), res


def kernel(x, W, r):
    out, _ = _run(x, W, r)
    return out


def kernel_traced(x, W, r):
    out, res = _run(x, W, r, trace=True)
    return out, res



# revision 17
# speedup vs baseline: 1.1368x; 1.1368x over previous
"""BIDE forward kernel for Trainium2, 8-core data parallel over B.

Math: logit[b, v] = sum_h cos(zlo[b, lo(v), h] + zhi[b, hi(v), h]) where
  zlo = bits(lo) @ W[:, :8].T          (lo = v & 255)
  zhi = bits(hi) @ W[:, 8:].T + r      (hi = v >> 8)
Using cos(p+q) = cos p cos q - sin p sin q, the [256, 256] logits table is
two K=128 matmuls over trig tables of shape [128 h, 256]:
  table = CloT.T @ ChiT - SloT.T @ ShiT   (per batch row)
The PSUM layout is table[hi, lo] -> t_ps[hi % 128, 256*(hi>>7) + lo].

Gather: instead of a DRAM round-trip + per-element indirect DMA (the old
bottleneck: 8 SWDGE calls ~1.9us each on Pool plus SBUF-port-serialized
4B writes), x is bucketed on the host by bits 4-6 of hi -- exactly the
16-partition GROUP of the table's natural layout -- and gathered on-chip
with one gpsimd ap_gather per batch row: group a fetches column
m = 256*(hi>>7) + lo for each of its t's across its 16 partitions
(hi%16 = partition within group). The 16-way partition ambiguity is
resolved by a host-shipped 0/1 mask multiply and a [128 -> 8] selection
matmul. Bucket assignment, padding, and output permutation are host-side
index prep; logZ's exp+sum runs on device, the final log and subtract
are host epilogue.

Range reduction for Sin (only valid on [-pi, pi]) is one DVE op per trig
table: w = (q + 8.5 or 8.75) mod 1.0 in [0, 1) where q = z/2pi (weights
pre-scaled by 1/2pi; +0.25 rotates sin into cos), then
Sin(2pi*w - pi) = sin(z) / cos(z), and Sin(-2pi*w + pi) = -sin(z).

Each core handles 2 of the 16 batch rows; zero cross-core communication.
"""

import numpy as np
import ml_dtypes
from contextlib import ExitStack

import concourse.bacc as bacc
import concourse.bass as bass  # noqa: F401
from concourse import mybir
from concourse.bass_utils import run_bass_kernel_spmd
from concourse.tile import TileContext

F32 = mybir.dt.float32
F32R = mybir.dt.float32r
BF16 = mybir.dt.bfloat16
I16 = mybir.dt.int16

PI = float(np.float32(np.pi))
TWO_PI = float(np.float32(2.0 * np.pi))
INV_2PI = 1.0 / (2.0 * np.pi)
# logits for these inputs peak at ~89 (exp overflows fp32); shift exp by a
# constant, added back in the host-side log
EXP_SHIFT = 60.0

N_CORES = 8
B, H, T = 16, 128, 4096
BPC = B // N_CORES  # batch rows per core (2)


def _build(nv):
    """nv = padded per-group gather count (multiple of 64)."""
    iw = nv // 16  # idx columns per batch row (int16, 16-partition wrap)
    nvh = nv // 2
    nc = bacc.Bacc("TRN2", target_bir_lowering=False, debug=False)

    # lhsT for the z matmuls (cols 0-511), one 128-col group per (b, half):
    # rows 0-7 W_hi bits, 8-15 W_lo residual, 16 r_hi, 17 r_lo (hi half
    # only); cols 512-767 bit-plane enumeration of v in [0, 256)
    wb = nc.dram_tensor("wb", [18, 768], BF16, kind="ExternalInput")
    # per-group gather columns, wrapped: idx i of group a lives at
    # partition 16a + i%16, col i//16 (+ iw per batch row)
    gidx = nc.dram_tensor("gidx", [128, 2 * iw], I16, kind="ExternalInput")
    # 16-way partition-selection mask: mask[16a+e, nv*b + i] = (e == hi%16);
    # cols [2nv, 2nv+8) hold the selection lhsT S[p, a] = (p//16 == a)
    maskd = nc.dram_tensor("maskd", [128, 2 * nv + 8], BF16, kind="ExternalInput")
    # row a = bucket, col b*nv + i = slot (matmul out base partition must be
    # 0/32/64, so both batch rows live on partitions 0-7, split by column)
    outp = nc.dram_tensor("outp", [8, 2 * nv], F32, kind="ExternalOutput")
    sums = nc.dram_tensor("sums", [128, 2], F32, kind="ExternalOutput")

    with ExitStack() as ctx:
        tc = ctx.enter_context(TileContext(nc))
        sb = ctx.enter_context(tc.tile_pool(name="sb", bufs=1))
        ps_z = ctx.enter_context(tc.tile_pool(name="ps_z", bufs=2, space="PSUM"))
        ps_t = ctx.enter_context(tc.tile_pool(name="ps_t", bufs=2, space="PSUM"))
        ps_r = ctx.enter_context(tc.tile_pool(name="ps_r", bufs=1, space="PSUM"))

        # ---- input loads
        wb_sb = sb.tile([18, 768], BF16, tag="wb")
        gidx_sb = sb.tile([128, 2 * iw], I16, tag="gidx")
        mask_sb = sb.tile([128, 2 * nv + 8], BF16, tag="mask")
        nc.sync.dma_start(out=wb_sb[:], in_=wb[:])
        nc.sync.dma_start(out=gidx_sb[:], in_=gidx[:])
        nc.sync.dma_start(out=mask_sb[:], in_=maskd[:])
        wp_sb = wb_sb[:, 0:512]
        bits_sb = wb_sb[:, 512:768]

        # ---- constants (off critical path; const-AP registry only has 0/1
        # so the pi/2 bias needs a per-partition tile)
        pio2 = sb.tile([128, 1], F32, tag="pio2")
        nc.vector.memset(pio2[:], PI / 2.0)
        nshift = sb.tile([128, 1], F32, tag="nshift")
        nc.vector.memset(nshift[:], -EXP_SHIFT)
        # selection lhsT: S[p, a] = (p//16 == a), summing a group's 16
        # partitions (one nonzero after masking); host-shipped in maskd
        s_sb = mask_sb[:, 2 * nv : 2 * nv + 8]

        # ---- q matmuls: q = z/2pi (weights pre-scaled by 1/2pi), [b0|b1]
        qlo = ps_z.tile([128, 512], F32, tag="z")
        qhi = ps_z.tile([128, 512], F32, tag="z")
        for b in range(BPC):
            bs = slice(256 * b, 256 * b + 256)
            nc.tensor.matmul(
                out=qlo[:, bs],
                lhsT=wp_sb[:, 128 * (2 * b) : 128 * (2 * b) + 128],
                rhs=bits_sb[:],
                start=True,
                stop=True,
            )
            nc.tensor.matmul(
                out=qhi[:, bs],
                lhsT=wp_sb[:, 128 * (2 * b + 1) : 128 * (2 * b + 1) + 128],
                rhs=bits_sb[:],
                start=True,
                stop=True,
            )

        # ---- range reduction (qi = round(q + cp) via the rounding f32->i32
        # convert, w = q - qi) + Sin; cos = Sin(2pi*w' + pi/2) with the
        # rounding point shifted so the arg stays in [-pi, pi]
        I32 = mybir.dt.int32

        def wred(q_ps, cp, tag):
            qi = sb.tile([128, 512], I32, tag=f"qi{tag}")
            if cp == 0.0:
                nc.vector.tensor_copy(out=qi[:], in_=q_ps[:])
            else:
                nc.vector.tensor_scalar(
                    out=qi[:], in0=q_ps[:], scalar1=cp, scalar2=None,
                    op0=mybir.AluOpType.add,
                )
            w = sb.tile([128, 512], F32, tag=f"w{tag}")
            nc.vector.tensor_tensor(
                out=w[:], in0=q_ps[:], in1=qi[:], op=mybir.AluOpType.subtract
            )
            return w

        wcl = wred(qlo, 0.25, "cl")
        wch = wred(qhi, 0.25, "ch")
        wsl = wred(qlo, 0.0, "sl")
        wsh = wred(qhi, 0.0, "sh")

        t_a = sb.tile([128, 512], BF16, tag="t_a")  # cos(zlo)
        t_b = sb.tile([128, 512], BF16, tag="t_b")  # cos(zhi)
        t_c = sb.tile([128, 512], BF16, tag="t_c")  # sin(zlo)
        t_d = sb.tile([128, 512], BF16, tag="t_d")  # -sin(zhi)
        for t_t, w_t, scale, bias in (
            (t_a, wcl, TWO_PI, pio2),
            (t_b, wch, TWO_PI, pio2),
            (t_c, wsl, TWO_PI, 0.0),
            (t_d, wsh, -TWO_PI, 0.0),
        ):
            nc.scalar.activation(
                out=t_t[:], in_=w_t[:],
                func=mybir.ActivationFunctionType.Sin,
                bias=bias if isinstance(bias, float) else bias[:],
                scale=scale,
            )

        # ---- per-b: table matmuls -> (gather chain) || (exp+sum chain)
        sums2 = sb.tile([128, 2], F32, tag="sums2")
        r_ps = [
            [
                ps_r.tile([8, nvh], F32, tag=f"r{b}{c}", name=f"r_ps{b}{c}")
                for c in range(2)
            ]
            for b in range(BPC)
        ]
        t_ps = []
        t_sbs = []
        y_sb = []
        m_sb = []
        for b in range(BPC):
            bs = slice(256 * b, 256 * b + 256)
            tp = ps_t.tile([128, 512], F32, tag="tb")
            t_ps.append(tp)
            # table[hi, lo] = sum_h cos(zhi)cos(zlo) - sin(zhi)sin(zlo)
            for c in range(2):
                cs = slice(256 * c, 256 * c + 256)
                hi_s = slice(256 * b + 128 * c, 256 * b + 128 * c + 128)
                nc.tensor.matmul(
                    out=tp[:, cs], lhsT=t_b[:, hi_s], rhs=t_a[:, bs],
                    start=True, stop=False,
                )
                nc.tensor.matmul(
                    out=tp[:, cs], lhsT=t_d[:, hi_s], rhs=t_c[:, bs],
                    start=False, stop=True,
                )
            # gather source must be SBUF
            ts = sb.tile([128, 512], F32, tag=f"tsb{b}")
            t_sbs.append(ts)
            nc.vector.tensor_copy(out=ts[:], in_=tp[:])
            # on-chip gather: group a (partitions 16a..16a+15) fetches its
            # t's column m = 256*(hi>>7) + lo across all 16 partitions
            y = sb.tile([128, nv], F32, tag=f"y{b}")
            y_sb.append(y)
            nc.gpsimd.ap_gather(
                y[:], ts[:], gidx_sb[:, b * iw : b * iw + iw],
                channels=128, num_elems=512, d=1, num_idxs=nv,
            )

        for b in range(BPC):
            # keep only the hi%16 partition of each gathered column
            m = sb.tile([128, nv], BF16, tag=f"m{b}")
            m_sb.append(m)
            nc.vector.tensor_tensor(
                out=m[:], in0=y_sb[b][:],
                in1=mask_sb[:, b * nv : b * nv + nv],
                op=mybir.AluOpType.mult,
            )
            for c in range(2):
                nc.tensor.matmul(
                    out=r_ps[b][c][:], lhsT=s_sb[:],
                    rhs=m[:, nvh * c : nvh * c + nvh], start=True, stop=True,
                )
            # partition function: exp with shift (fp32 range) + fused row
            # sums via the ACT accumulator; host does log(sum) + shift
            e = sb.tile([128, 512], BF16, tag=f"e{b}")
            nc.scalar.activation(
                out=e[:], in_=t_ps[b][:],
                func=mybir.ActivationFunctionType.Exp, bias=nshift[:],
                accum_out=sums2[:, b : b + 1],
            )

        # ---- outputs (DMA cannot read PSUM: stage R in SBUF)
        r_sb = sb.tile([8, 2 * nv], F32, tag="r_sb")
        for b in range(BPC):
            for c in range(2):
                nc.vector.tensor_copy(
                    out=r_sb[:, nv * b + nvh * c : nv * b + nvh * c + nvh],
                    in_=r_ps[b][c][:],
                )
        nc.sync.dma_start(out=outp[:], in_=r_sb[:])
        nc.sync.dma_start(out=sums[:], in_=sums2[:])

    nc.finalize()
    return nc


_NC = {}


def _get_nc(nv):
    if nv not in _NC:
        _NC[nv] = _build(nv)
    return _NC[nv]


def _bf16_split(a):
    """Return (hi, lo) bf16 arrays with hi + lo ~= a (fp32)."""
    hi = a.astype(ml_dtypes.bfloat16)
    lo = (a - hi.astype(np.float32)).astype(ml_dtypes.bfloat16)
    return hi, lo


def _make_in_maps(x, W, r, nv):
    iw = nv // 16
    x = np.asarray(x, dtype=np.int32)
    W = np.asarray(W, dtype=np.float32)
    r = np.asarray(r, dtype=np.float32)

    v = np.arange(256, dtype=np.int32)
    k = np.arange(8, dtype=np.int32)
    bitplanes = ((v[None, :] >> k[:, None]) & 1).astype(np.float32)  # [8, 256]
    bits = np.ones((18, 256), dtype=np.float32)
    bits[0:8] = bitplanes
    bits[8:16] = bitplanes

    in_maps = []
    unpack = []  # per core: per b_loc (torder, arow, islot)
    for core in range(N_CORES):
        wb = np.zeros((18, 768), dtype=ml_dtypes.bfloat16)
        wb[:, 512:768] = bits.astype(ml_dtypes.bfloat16)
        gidx = np.zeros((128, 2 * iw), dtype=np.int16)
        mask = np.zeros((128, 2 * nv + 8), dtype=ml_dtypes.bfloat16)
        for a in range(8):
            mask[16 * a : 16 * a + 16, 2 * nv + a] = 1.0
        up = []
        for b_loc in range(BPC):
            b = BPC * core + b_loc
            for half in range(2):
                g = 2 * b_loc + half
                cs = slice(128 * g, 128 * g + 128)
                w_t = W[b, :, 8 * half : 8 * half + 8].T * INV_2PI  # [8, 128]
                w_hi, w_lo = _bf16_split(w_t.astype(np.float32))
                wb[0:8, cs] = w_hi
                wb[8:16, cs] = w_lo
                if half == 1:
                    r_hi, r_lo = _bf16_split((r[b] * INV_2PI).astype(np.float32))
                    wb[16, cs] = r_hi
                    wb[17, cs] = r_lo
            # bucket t's by bits 4-6 of hi (= 16-partition group of the
            # table layout); within-bucket slot i -> gather column i
            vx = x[b]
            lo = vx & 255
            hi = vx >> 8
            a = (hi >> 4) & 7
            e = hi & 15
            m = ((hi >> 7) & 1) * 256 + lo
            order = np.argsort(a, kind="stable")
            a_s = a[order]
            cnt = np.bincount(a_s, minlength=8)
            starts = np.concatenate(([0], np.cumsum(cnt)))[:-1]
            i_s = np.arange(T, dtype=np.int64) - starts[a_s]
            gidx[16 * a_s + (i_s % 16), b_loc * iw + i_s // 16] = m[order].astype(
                np.int16
            )
            mask[16 * a_s + e[order], b_loc * nv + i_s] = 1.0
            up.append((order, a_s, i_s))
        in_maps.append({"wb": wb, "gidx": gidx, "maskd": mask})
        unpack.append(up)
    return in_maps, unpack


def _nv_for(x):
    x = np.asarray(x, dtype=np.int32)
    hi = x >> 8
    a = (hi >> 4) & 7
    mx = 0
    for b in range(B):
        mx = max(mx, int(np.bincount(a[b], minlength=8).max()))
    return max(256, ((mx + 63) // 64) * 64)


def _run(x, W, r, trace=False):
    nv = _nv_for(x)
    nc = _get_nc(nv)
    in_maps, unpack = _make_in_maps(x, W, r, nv)
    res = run_bass_kernel_spmd(nc, in_maps, core_ids=list(range(N_CORES)), trace=trace)
    out = np.empty((B, T), dtype=np.float32)
    for core in range(N_CORES):
        rtab = np.asarray(res.results[core]["outp"], dtype=np.float32)  # [8, 2nv]
        s2 = np.asarray(res.results[core]["sums"], dtype=np.float32)  # [128, 2]
        nv = rtab.shape[1] // 2
        for b_loc in range(BPC):
            b = BPC * core + b_loc
            order, a_s, i_s = unpack[core][b_loc]
            logz = np.float32(np.log(s2[:, b_loc].sum()) + EXP_SHIFT)
            out[b, order] = rtab[a_s, b_loc * nv + i_s] - logz
    return out, res


def kernel(x, W, r):
    out, _ = _run(x, W, r)
    return out


def kernel_traced(x, W, r):
    out, res = _run(x, W, r, trace=True)
    return out, res


# revision 24
# speedup vs baseline: 1.1932x; 1.0496x over previous
"""BIDE forward kernel for Trainium2, 8-core data parallel over B.

Math: logit[b, v] = sum_h cos(zlo[b, lo(v), h] + zhi[b, hi(v), h]) where
  zlo = bits(lo) @ W[:, :8].T          (lo = v & 255)
  zhi = bits(hi) @ W[:, 8:].T + r      (hi = v >> 8)
Using cos(p+q) = cos p cos q - sin p sin q, the [256, 256] logits table is
two K=128 matmuls over bf16 trig tables of shape [128 h, 256]:
  table = CloT.T @ ChiT - SloT.T @ ShiT   (per batch row)
tbl[v] in DRAM has flat index v = 256*hi + lo = x, so the gather is ONE
SWDGE indirect DMA per batch row: dest g[128, 32] spread over all 128
partitions (every SBUF write port; no redistribution -- the offset AP and
dest AP walk in the same partition-fastest order i = 128s + p, host packs
xv[p, 32b+s] = x[b, 128s+p]). One call per row keeps the serial SWDGE
descriptor-gen on Pool to 2 x ~2.4us instead of 8 x ~1.9us.

Range reduction for Sin (only valid on [-pi, pi]): qi = round(q + cp) via
the rounding f32->i32 convert, w = q - qi, with q = z/2pi (weights
pre-scaled by 1/2pi). cp=0 for sin (Sin(2pi w)), cp=0.25 for cos
(Sin(2pi w + pi/2), arg in [-pi, pi]); -sin via scale=-2pi.

logZ: Exp(table - 60) with the fused ACT row-sum accumulator; host does
log(sum) + 60 and the final subtract while unpermuting.

Each core handles 2 of the 16 batch rows; zero cross-core communication.
"""

import numpy as np
import ml_dtypes
from contextlib import ExitStack

import concourse.bacc as bacc
import concourse.bass as bass
from concourse import mybir
from concourse.bass_utils import run_bass_kernel_spmd
from concourse.tile import TileContext

F32 = mybir.dt.float32
BF16 = mybir.dt.bfloat16
I32 = mybir.dt.int32

PI = float(np.float32(np.pi))
TWO_PI = float(np.float32(2.0 * np.pi))
INV_2PI = 1.0 / (2.0 * np.pi)
# logits for these inputs peak at ~89 (exp overflows fp32); shift exp by a
# constant, added back in the host-side log
EXP_SHIFT = 60.0

N_CORES = 8
B, H, T = 16, 128, 4096
BPC = B // N_CORES  # batch rows per core (2)


def _build():
    nc = bacc.Bacc("TRN2", target_bir_lowering=False, debug=False)

    # lhsT for the z matmuls (cols 0-511), one 128-col group per (b, half):
    # rows 0-7 W_hi bits, 8-15 W_lo residual, 16 r_hi, 17 r_lo (hi half
    # only); cols 512-767 bit-plane enumeration of v in [0, 256)
    wb = nc.dram_tensor("wb", [18, 768], BF16, kind="ExternalInput")
    # gather offsets: xv[p, 32b + j] = x[b, 128j + p]
    xv = nc.dram_tensor("xv", [128, 64], I32, kind="ExternalInput")
    # row 8b + s = t block [1024s, 1024s + 1024) of batch row b
    outp = nc.dram_tensor("outp", [8, 1024], F32, kind="ExternalOutput")
    sums = nc.dram_tensor("sums", [128, 2], F32, kind="ExternalOutput")

    with ExitStack() as ctx:
        tc = ctx.enter_context(TileContext(nc))
        sb = ctx.enter_context(tc.tile_pool(name="sb", bufs=1))
        ps_z = ctx.enter_context(tc.tile_pool(name="ps_z", bufs=2, space="PSUM"))
        ps_t = ctx.enter_context(tc.tile_pool(name="ps_t", bufs=2, space="PSUM"))
        dram = ctx.enter_context(tc.tile_pool(name="dram", bufs=1, space="DRAM"))

        # ---- input loads (wb from the Pool queue: Sync's init drain delays
        # its first HWDGE dispatch, and the z matmuls gate on wb)
        wb_sb = sb.tile([18, 768], BF16, tag="wb")
        xv_sb = sb.tile([128, 64], I32, tag="xv")
        nc.gpsimd.dma_start(out=wb_sb[:], in_=wb[:])
        nc.sync.dma_start(out=xv_sb[:], in_=xv[:])
        wp_sb = wb_sb[:, 0:512]
        bits_sb = wb_sb[:, 512:768]

        # ---- constants (off critical path; const-AP registry only has 0/1
        # so the pi/2 bias needs a per-partition tile)
        pio2 = sb.tile([128, 1], F32, tag="pio2")
        nc.vector.memset(pio2[:], PI / 2.0)
        nshift = sb.tile([128, 1], F32, tag="nshift")
        nc.vector.memset(nshift[:], -EXP_SHIFT)

        # ---- q matmuls: q = z/2pi (weights pre-scaled by 1/2pi), [b0|b1]
        qlo = ps_z.tile([128, 512], F32, tag="z")
        qhi = ps_z.tile([128, 512], F32, tag="z")
        for b in range(BPC):
            bs = slice(256 * b, 256 * b + 256)
            nc.tensor.matmul(
                out=qlo[:, bs],
                lhsT=wp_sb[:, 128 * (2 * b) : 128 * (2 * b) + 128],
                rhs=bits_sb[:],
                start=True,
                stop=True,
            )
            nc.tensor.matmul(
                out=qhi[:, bs],
                lhsT=wp_sb[:, 128 * (2 * b + 1) : 128 * (2 * b + 1) + 128],
                rhs=bits_sb[:],
                start=True,
                stop=True,
            )

        # ---- range reduction + Sin
        def wred(q_ps, cp, tag):
            qi = sb.tile([128, 512], I32, tag=f"qi{tag}")
            if cp == 0.0:
                nc.vector.tensor_copy(out=qi[:], in_=q_ps[:])
            else:
                nc.vector.tensor_scalar(
                    out=qi[:], in0=q_ps[:], scalar1=cp, scalar2=None,
                    op0=mybir.AluOpType.add,
                )
            w = sb.tile([128, 512], F32, tag=f"w{tag}")
            nc.vector.tensor_tensor(
                out=w[:], in0=q_ps[:], in1=qi[:], op=mybir.AluOpType.subtract
            )
            return w

        wcl = wred(qlo, 0.25, "cl")
        wch = wred(qhi, 0.25, "ch")
        wsl = wred(qlo, 0.0, "sl")
        wsh = wred(qhi, 0.0, "sh")

        t_a = sb.tile([128, 512], BF16, tag="t_a")  # cos(zlo)
        t_b = sb.tile([128, 512], BF16, tag="t_b")  # cos(zhi)
        t_c = sb.tile([128, 512], BF16, tag="t_c")  # sin(zlo)
        t_d = sb.tile([128, 512], BF16, tag="t_d")  # -sin(zhi)
        for t_t, w_t, scale, bias in (
            (t_a, wcl, TWO_PI, pio2),
            (t_b, wch, TWO_PI, pio2),
            (t_c, wsl, TWO_PI, 0.0),
            (t_d, wsh, -TWO_PI, 0.0),
        ):
            nc.scalar.activation(
                out=t_t[:], in_=w_t[:],
                func=mybir.ActivationFunctionType.Sin,
                bias=bias if isinstance(bias, float) else bias[:],
                scale=scale,
            )

        # ---- per-b: table matmuls -> DRAM table -> indirect gather, with
        # the exp+sum (partition function) on the ACT in parallel
        sums2 = sb.tile([128, 2], F32, tag="sums2")
        # gather dest: the SWDGE walks only the dest's base partition, so one
        # call per strip; partitions {4s + 16b} map to the 8 distinct SBUF
        # write ports (port = p//4 mod 8), transfers run port-parallel
        g_sb = sb.tile([32, 1024], F32, tag="g")
        for b in range(BPC):
            bs = slice(256 * b, 256 * b + 256)
            tp = ps_t.tile([128, 512], F32, tag="tb", name=f"tp{b}")
            # table[hi, lo] = sum_h cos(zhi)cos(zlo) - sin(zhi)sin(zlo)
            for c in range(2):
                cs = slice(256 * c, 256 * c + 256)
                hi_s = slice(256 * b + 128 * c, 256 * b + 128 * c + 128)
                nc.tensor.matmul(
                    out=tp[:, cs], lhsT=t_b[:, hi_s], rhs=t_a[:, bs],
                    start=True, stop=False,
                )
                nc.tensor.matmul(
                    out=tp[:, cs], lhsT=t_d[:, hi_s], rhs=t_c[:, bs],
                    start=False, stop=True,
                )
            # DMA cannot read PSUM: stage in SBUF, then to DRAM per c-half
            # so the tbl write overlaps the c1 matmuls
            ts = sb.tile([128, 512], F32, tag=f"tsb{b}")
            tbl = dram.tile([65536, 1], F32, tag=f"tbl{b}", name=f"tbl{b}")
            for c in range(2):
                cs = slice(256 * c, 256 * c + 256)
                nc.vector.tensor_copy(out=ts[:, cs], in_=tp[:, cs])
                dst = tbl[32768 * c : 32768 * (c + 1), 0:1].rearrange(
                    "(p n) one -> p (n one)", p=128
                )
                nc.sync.dma_start(out=dst, in_=ts[:, cs])
            # gather: 4 strips per batch row; offsets walk partition-major
            # (i = 128c + p over the [128, 8] slice), so strip s covers
            # t in [1024s, 1024s + 1024)
            for s in range(4):
                row = 4 * s + 16 * b
                nc.gpsimd.indirect_dma_start(
                    out=g_sb[row : row + 1, :].rearrange(
                        "one (i x) -> one i x", x=1
                    ),
                    out_offset=None,
                    in_=tbl[:],
                    in_offset=bass.IndirectOffsetOnAxis(
                        ap=xv_sb[:, 32 * b + 8 * s : 32 * b + 8 * s + 8], axis=0
                    ),
                )
            # partition function: exp with shift (fp32 range) + fused row
            # sums via the ACT accumulator; host does log(sum) + shift
            e = sb.tile([128, 512], BF16, tag=f"e{b}")
            nc.scalar.activation(
                out=e[:], in_=tp[:],
                func=mybir.ActivationFunctionType.Exp, bias=nshift[:],
                accum_out=sums2[:, b : b + 1],
            )

        # ---- outputs: the 8 strip partitions {0,4,...,28}, stride-4 AP
        nc.sync.dma_start(out=outp[:], in_=g_sb[0:32:4, :])
        nc.sync.dma_start(out=sums[:], in_=sums2[:])

    nc.finalize()
    return nc


_NC = None


def _get_nc():
    global _NC
    if _NC is None:
        _NC = _build()
    return _NC


def _bf16_split(a):
    """Return (hi, lo) bf16 arrays with hi + lo ~= a (fp32)."""
    hi = a.astype(ml_dtypes.bfloat16)
    lo = (a - hi.astype(np.float32)).astype(ml_dtypes.bfloat16)
    return hi, lo


def _make_in_maps(x, W, r):
    x = np.asarray(x, dtype=np.int32)
    W = np.asarray(W, dtype=np.float32)
    r = np.asarray(r, dtype=np.float32)

    v = np.arange(256, dtype=np.int32)
    k = np.arange(8, dtype=np.int32)
    bitplanes = ((v[None, :] >> k[:, None]) & 1).astype(np.float32)  # [8, 256]
    bits = np.ones((18, 256), dtype=np.float32)
    bits[0:8] = bitplanes
    bits[8:16] = bitplanes

    in_maps = []
    for core in range(N_CORES):
        wb = np.zeros((18, 768), dtype=ml_dtypes.bfloat16)
        wb[:, 512:768] = bits.astype(ml_dtypes.bfloat16)
        xvs = []
        for b_loc in range(BPC):
            b = BPC * core + b_loc
            for half in range(2):
                g = 2 * b_loc + half
                cs = slice(128 * g, 128 * g + 128)
                w_t = W[b, :, 8 * half : 8 * half + 8].T * INV_2PI  # [8, 128]
                w_hi, w_lo = _bf16_split(w_t.astype(np.float32))
                wb[0:8, cs] = w_hi
                wb[8:16, cs] = w_lo
                if half == 1:
                    r_hi, r_lo = _bf16_split((r[b] * INV_2PI).astype(np.float32))
                    wb[16, cs] = r_hi
                    wb[17, cs] = r_lo
            xvs.append(x[b].reshape(32, 128).T)
        in_maps.append(
            {"wb": wb, "xv": np.concatenate(xvs, axis=1).astype(np.int32)}
        )
    return in_maps


def _run(x, W, r, trace=False):
    nc = _get_nc()
    in_maps = _make_in_maps(x, W, r)
    res = run_bass_kernel_spmd(nc, in_maps, core_ids=list(range(N_CORES)), trace=trace)
    out = np.empty((B, T), dtype=np.float32)
    for core in range(N_CORES):
        g = np.asarray(res.results[core]["outp"], dtype=np.float32)  # [8, 1024]
        s2 = np.asarray(res.results[core]["sums"], dtype=np.float32)  # [128, 2]
        for b_loc in range(BPC):
            b = BPC * core + b_loc
            logz = np.float32(np.log(s2[:, b_loc].sum()) + EXP_SHIFT)
            out[b] = g[4 * b_loc : 4 * b_loc + 4, :].reshape(T) - logz
    return out, res


def kernel(x, W, r):
    out, _ = _run(x, W, r)
    return out


def kernel_traced(x, W, r):
    out, res = _run(x, W, r, trace=True)
    return out, res
